# revision 24
# baseline (speedup 1.0000x reference)
"""Bass/Trainium2 kernel for nn_AttentionDispatcher (region-dispatch MHA).

Sharding: 8 cores = 4 ops x 2 batch-pairs. Each core runs full MHA for one
(src-region -> dst-region) op over 2 batches. Gathers/scatter-adds happen on
host (cheap numpy fancy indexing); the device does only dense attention math.
"""

import sys

sys.path.insert(0, "/opt/trn_rl_repo")

import numpy as np
import ml_dtypes

from concourse import bacc
from concourse import library_config
import concourse.mybir as mybir
from concourse.tile import TileContext
from concourse.bass_utils import run_bass_kernel_spmd

B, N, D, H = 4, 4096, 512, 8
DH = D // H          # 64 head dim
S = 1024             # region size (q rows and kv rows per op)
OPS = [(0, 1, 1.0), (0, 2, 0.5), (1, 3, 1.0), (2, 0, 1.0)]
NB = 2               # batches per core
P = 128
DC = D // P          # 4 contraction chunks over D
JC = D // P          # 4 chunks over projected dim
TC = S // P          # 8 chunks over kv rows
NIH = 2              # halves of the 1024 q rows (free dim 512)
HW = 512
F32 = mybir.dt.float32
BF16 = mybir.dt.bfloat16
BF = ml_dtypes.bfloat16
EXPF = mybir.ActivationFunctionType.Exp
MULT = mybir.AluOpType.mult

_CACHE = {}


def _build():
    nc = bacc.Bacc()
    qT_e = nc.declare_dram_parameter("qT", [NB, D, S], BF16, isOutput=False)
    kvT_e = nc.declare_dram_parameter("kvT", [NB, D, S], BF16, isOutput=False)
    wq_e = nc.declare_dram_parameter("wq", [D, D], BF16, isOutput=False)
    wk_e = nc.declare_dram_parameter("wk", [D, D], BF16, isOutput=False)
    wv_e = nc.declare_dram_parameter("wv", [D, D], BF16, isOutput=False)
    wo_e = nc.declare_dram_parameter("wo", [D, D], BF16, isOutput=False)
    out_e = nc.declare_dram_parameter("out", [NB, S, D], F32, isOutput=True)

    with TileContext(nc) as tc:
        with (
            tc.tile_pool(name="const", bufs=1) as constp,
            tc.tile_pool(name="big", bufs=1) as bigp,
            tc.tile_pool(name="expp", bufs=2) as expp,
            tc.tile_pool(name="small", bufs=3) as smallp,
            tc.tile_pool(name="ops", bufs=2) as opsp,
            tc.tile_pool(name="mm_ps", bufs=2, space="PSUM") as mmp,
            tc.tile_pool(name="sc_ps", bufs=2, space="PSUM") as scp,
            tc.tile_pool(name="av_ps", bufs=1, space="PSUM") as avp,
        ):
            # ---- weights to SBUF: [d_in=128, d_chunk, j] ----
            wsb = {}
            for nm, ext in (("wk", wk_e), ("wq", wq_e), ("wv", wv_e), ("wo", wo_e)):
                t = constp.tile([P, DC, D], BF16, name=f"sb_{nm}", tag=f"sb_{nm}")
                nc.sync.dma_start(t[:], ext.rearrange("(dc p) j -> p dc j", p=P))
                wsb[nm] = t
            nc.gpsimd.load_library(library_config.attn)

            qt, kvt, qpT, kpT, vp, att = {}, {}, {}, {}, {}, {}
            for b in range(NB):
                qt[b] = bigp.tile([P, DC, S], BF16, name=f"qt{b}", tag=f"qt{b}")
                kvt[b] = bigp.tile([P, DC, S], BF16, name=f"kvt{b}", tag=f"kvt{b}")
                qpT[b] = bigp.tile([P, JC, S], BF16, name=f"qpT{b}", tag=f"qpT{b}")
                kpT[b] = bigp.tile([P, JC, S], BF16, name=f"kpT{b}", tag=f"kpT{b}")
                vp[b] = bigp.tile([P, TC, H, DH + 1], BF16, name=f"vp{b}", tag=f"vp{b}")
                att[b] = bigp.tile([P, JC, S], BF16, name=f"att{b}", tag=f"att{b}")
            for b in range(NB):  # kv first (kpT then vp groups lead)
                for dc in range(DC):  # chunked so first matmuls start early
                    nc.sync.dma_start(
                        kvt[b][:, dc, :],
                        kvT_e[b].rearrange("(dc p) i -> p dc i", p=P)[:, dc, :])
                    nc.sync.dma_start(
                        qt[b][:, dc, :],
                        qT_e[b].rearrange("(dc p) i -> p dc i", p=P)[:, dc, :])

            # ---- projection group emitters (deferred for interleaving) ----
            def qk_proj_group(b, wname, jc, ih):
                def emit():
                    dst = qpT[b] if wname == "wq" else kpT[b]
                    src = qt[b] if wname == "wq" else kvt[b]
                    ps = mmp.tile([P, HW], F32, name="mmps", tag="mmps")
                    for dc in range(DC):
                        nc.tensor.matmul(
                            ps[:],
                            wsb[wname][:, dc, jc * P:(jc + 1) * P],
                            src[:, dc, ih * HW:(ih + 1) * HW],
                            start=(dc == 0), stop=(dc == DC - 1))
                    nc.vector.tensor_copy(dst[:, jc, ih * HW:(ih + 1) * HW], ps[:])
                return emit

            def vp_group(b, tcc):
                def emit():
                    ps = mmp.tile([P, HW], F32, name="mmps", tag="mmps")
                    for dc in range(DC):
                        nc.tensor.matmul(
                            ps[:],
                            kvt[b][:, dc, tcc * P:(tcc + 1) * P],
                            wsb["wv"][:, dc, :],
                            start=(dc == 0), stop=(dc == DC - 1))
                    nc.vector.tensor_copy(
                        vp[b][:, tcc, :, 0:DH],
                        ps.rearrange("p (h d) -> p h d", h=H))
                return emit

            def oproj_group(b, ic):
                def emit():
                    ps = mmp.tile([P, HW], F32, name="mmps", tag="mmps")
                    for jc in range(JC):
                        nc.tensor.matmul(
                            ps[:],
                            att[b][:, jc, ic * P:(ic + 1) * P],
                            wsb["wo"][:, jc, :],
                            start=(jc == 0), stop=(jc == JC - 1))
                    o_sb = opsp.tile([P, HW], F32, name="osb", tag="osb")
                    nc.vector.tensor_copy(o_sb[:], ps[:])
                    nc.sync.dma_start(
                        out_e[b].rearrange("(ic p) m -> p ic m", p=P)[:, ic, :],
                        o_sb[:])
                return emit

            # warm the ACT exp table during the startup DMA wait
            warm = smallp.tile([1, 1], F32, name="warm", tag="warm")
            nc.vector.memset(warm[:], 0.0)
            nc.scalar.activation(warm[:], warm[:], EXPF)

            # up-front: minimum needed to start attention (b0, hp0)
            nc.vector.memset(vp[0][:, :, :, DH:DH + 1], 1.0)
            nc.vector.memset(vp[1][:, :, :, DH:DH + 1], 1.0)
            for ih in range(NIH):
                qk_proj_group(0, "wk", 0, ih)()
                qk_proj_group(0, "wq", 0, ih)()
            for tcc in range(4):
                vp_group(0, tcc)()

            # fillers sprinkled into PE-idle slots of the ACT-bound middles.
            # Each carries the global tc-slot of its first consumer; it MUST
            # be emitted (= placed earlier in engine program order) before
            # that slot, with margin. Rate-pumping spreads the rest evenly.
            def gslot(b, hp, ih, tcc=0):
                return ((b * (H // 2) + hp) * NIH + ih) * TC + tcc

            fill_sched = []
            for tcc in range(4, TC):
                fill_sched.append((gslot(0, 0, 0, tcc), vp_group(0, tcc)))
            for bb in range(NB):
                for jc in range(1, JC) if bb == 0 else range(JC):
                    for ih in range(NIH):
                        dl = gslot(bb, jc, ih)
                        fill_sched.append((dl, qk_proj_group(bb, "wq", jc, ih)))
                        fill_sched.append((dl, qk_proj_group(bb, "wk", jc, ih)))
            for tcc in range(TC):
                fill_sched.append((gslot(1, 0, 0, tcc), vp_group(1, tcc)))
            big = 10 ** 6
            oproj_b0 = [(big, oproj_group(0, ic)) for ic in range(S // P)]

            # ---- attention middle (+ interleaved fillers) ----
            nslots_all = NB * (H // 2) * NIH * TC
            n_fill = len(fill_sched) + len(oproj_b0)
            fill_sched.sort(key=lambda t: t[0])
            fl = fill_sched[::-1]  # pop() from the deadline-ordered front
            n_emitted = 0
            MARGIN = 6
            for b in range(NB):
                if b == 1:
                    # b0 out-proj becomes eligible once b0's heads finish
                    fl = (fl[::-1] + oproj_b0)[::-1]
                for hp in range(H // 2):
                    for ih in range(NIH):
                        exp_t = expp.tile([P, TC, 2, HW], BF16, name="exps", tag="exps")
                        avs = [
                            avp.tile([DH + 1, HW], F32, name=f"avps{hh}", tag=f"avps{hh}")
                            for hh in range(2)
                        ]

                        def av_mms(tcc):
                            for hh in range(2):
                                nc.tensor.matmul(
                                    avs[hh][:],
                                    vp[b][:, tcc, 2 * hp + hh, :],
                                    exp_t[:, tcc, hh, :],
                                    start=(tcc == 0), stop=(tcc == TC - 1))

                        # software pipeline: scores(t) | exp(t) on ACT | av(t-1)
                        for tcc in range(TC):
                            sc = scp.tile([P, 2, HW], F32, name="scps", tag="scps")
                            for hh in range(2):  # row-tiled pair (base 0 / 64)
                                hlo = hh * 64
                                nc.tensor.matmul(
                                    sc[:, hh, :],
                                    kpT[b][hlo:hlo + 64, hp, tcc * P:(tcc + 1) * P],
                                    qpT[b][hlo:hlo + 64, hp, ih * HW:(ih + 1) * HW],
                                    start=True, stop=True)
                            nc.scalar.activation(
                                exp_t[:, tcc, :, :], sc[:], EXPF, scale=0.125)
                            if tcc > 0:
                                av_mms(tcc - 1)
                            cur = gslot(b, hp, ih, tcc)
                            rate_target = (n_fill * (cur + 1)) // nslots_all
                            while fl and (fl[-1][0] <= cur + MARGIN
                                          or n_emitted < rate_target):
                                fl.pop()[1]()
                                n_emitted += 1
                        av_mms(TC - 1)
                        for hh in range(2):
                            rec = smallp.tile([1, HW], F32, name="rec", tag="rec")
                            nc.vector.reciprocal(rec[:], avs[hh][DH:DH + 1, :])
                            bc_sb = smallp.tile([64, HW], F32, name="bcs", tag="bcs")
                            nc.gpsimd.partition_broadcast(bc_sb[:], rec[:])
                            nc.vector.tensor_tensor(
                                att[b][hh * 64:hh * 64 + 64, hp, ih * HW:(ih + 1) * HW],
                                avs[hh][0:DH, :], bc_sb[:], MULT)
                        if b == NB - 1 and hp == H // 2 - 1:
                            while fl:  # flush any stranded fillers
                                fl.pop()[1]()
                            # last batch out-proj: each i-half as soon as all
                            # heads' normalize for that half lands
                            ics = range(0, 4) if ih == 0 else range(4, S // P)
                            for ic in ics:
                                oproj_group(b, ic)()
    nc.compile()
    return nc


def _get_nc():
    if "nc" not in _CACHE:
        _CACHE["nc"] = _build()
    return _CACHE["nc"]


def _make_in_maps(x, idxs, Wq, Wk, Wv, Wo):
    xbf = np.asarray(x, np.float32).astype(BF)
    wqb = np.ascontiguousarray(np.asarray(Wq, np.float32).astype(BF))
    wkb = np.ascontiguousarray(np.asarray(Wk, np.float32).astype(BF))
    wvb = np.ascontiguousarray(np.asarray(Wv, np.float32).astype(BF))
    wo32 = np.asarray(Wo, np.float32)
    in_maps = []
    for c in range(8):
        o, p = divmod(c, 2)
        s, d, w = OPS[o]
        si, di = idxs[s], idxs[d]
        bs = (2 * p, 2 * p + 1)
        qT = np.stack([np.ascontiguousarray(xbf[b][si].T) for b in bs])
        kvT = np.stack([np.ascontiguousarray(xbf[b][di].T) for b in bs])
        wob = np.ascontiguousarray((wo32 * w).astype(BF))
        in_maps.append({"qT": qT, "kvT": kvT,
                        "wq": wqb, "wk": wkb, "wv": wvb, "wo": wob})
    return in_maps


def _run(in_maps, **kw):
    nc = _get_nc()
    return run_bass_kernel_spmd(nc, in_maps, core_ids=list(range(8)), **kw)


def kernel(x, idx0, idx1, idx2, idx3, Wq, Wk, Wv, Wo):
    x = np.asarray(x, np.float32)
    idxs = [np.asarray(i, np.int64) for i in (idx0, idx1, idx2, idx3)]
    in_maps = _make_in_maps(x, idxs, Wq, Wk, Wv, Wo)
    res = _run(in_maps).results

    acc = np.zeros_like(x)
    wmap = np.zeros(N, np.float32)
    for o, (s, d, w) in enumerate(OPS):
        si = idxs[s]
        uniq = np.unique(si).size == si.size
        for p in range(2):
            r = np.asarray(res[o * 2 + p]["out"], np.float32)
            for u, b in enumerate((2 * p, 2 * p + 1)):
                if uniq:
                    acc[b, si] += r[u]
                else:
                    np.add.at(acc, (b, si), r[u])
        np.add.at(wmap, si, np.float32(w))
    nz = wmap > 0
    denom = np.where(nz, wmap, 1.0).astype(np.float32)
    out = np.where(nz[None, :, None], acc / denom[None, :, None], x)
    return out.astype(np.float32)
